# revision 12
# baseline (speedup 1.0000x reference)
"""Trainium2 Bass kernel for nn_MeshLoss2D (chamfer min-distance mesh loss).

Computation: refine a (B,3,32,32) mesh grid by bilinear factor 3 to (B,3,94,94),
then for every point-cloud point (B,3,4096) find min squared distance to any
refined mesh point, and return the mean over all B*4096 points.

Sharding: 8 cores = (batch b, pc half h); each core handles 2048 pc points of
one batch and that batch's full mesh (8836 points, padded to 9216).

Device algorithm per core:
  - refine mesh via two fp32 matmuls with host-built interpolation matrix R^T
    (refined = Ry @ G_c @ Rx^T done as (G^T@Ry^T)^T stages on the PE).
  - build augmented fp16 hi/lo split operands:
      a = [p, 1]            (lhsT rows:  [a_hi, a_hi, a_lo]  -> 12 partitions)
      b = [-2m, ||m||^2]    (rhs  rows:  [b_hi, b_lo, b_hi])
    so   a.T b = -2 p.m + ||m||^2 = d(p,m) - ||p||^2   (exact to ~2^-22)
  - PE: for each 128-pc-point tile, stream 18 N-chunks of 512 mesh points into
    PSUM (fp32).
  - DVE: tensor_tensor_reduce(op0=min, op1=min) consumes PSUM tile pairs and
    chains a running per-partition min -> [128,16] results.
Host: final = mean(minaug + ||p||^2)  (mean of ||p||^2 added back on host).
"""

import os
import sys

for _p in ("/opt/trn_rl_repo", "/opt/trn_rl_repo/concourse"):
    if _p not in sys.path:
        sys.path.insert(0, _p)

import numpy as np

B, C, H, W = 4, 3, 32, 32
FACTOR = 3
OH = (H - 1) * FACTOR + 1        # 94
N_MESH = OH * OH                 # 8836
N_PAD = 9216                     # 18 chunks of 512
N_CHUNKS = N_PAD // 512          # 18
M_TOTAL = 4096
N_CORES = 8
M_CORE = M_TOTAL * B // N_CORES  # 2048 pc points per core
PC_TILES = M_CORE // 128         # 16
PAD_BIG = 60000.0                # sentinel ||m||^2 for padded mesh points
DIRECT_QUADS = {0}               # quad-groups reduced directly from PSUM

_BUILT = {}
LAST_RESULTS = None


def _interp_matrix():
    """R [OH, H] fp32 with R[o, y0]=1-w, R[o, y0+1]=w replicating reference
    fp32 arithmetic (ys = arange(oh)/3 in fp32)."""
    ys = np.arange(OH, dtype=np.float32) / np.float32(FACTOR)
    y0 = np.clip(np.floor(ys).astype(np.int64), 0, H - 2)
    wy = ys - y0.astype(np.float32)
    R = np.zeros((OH, H), dtype=np.float32)
    R[np.arange(OH), y0] = np.float32(1.0) - wy
    R[np.arange(OH), y0 + 1] += wy
    return R


def _build_kernel():
    from concourse import bacc, mybir
    import concourse.tile as tile

    f32 = mybir.dt.float32
    f16 = mybir.dt.float16
    MIN = mybir.AluOpType.min
    MULT = mybir.AluOpType.mult
    SUB = mybir.AluOpType.subtract
    ADD = mybir.AluOpType.add

    nc = bacc.Bacc(
        "TRN2",
        target_bir_lowering=False,
        debug=False,
        enable_asserts=False,
        num_devices=N_CORES,
    )

    grid = nc.dram_tensor("mesh_grid", (C, H, W), f32, kind="ExternalInput").ap()
    pcs = nc.dram_tensor("a_aug", (12, M_CORE), f16, kind="ExternalInput").ap()
    bpad = nc.dram_tensor("b_pad", (12, N_PAD - N_MESH), f16, kind="ExternalInput").ap()
    rmat = nc.dram_tensor("rmat", (H, OH), f32, kind="ExternalInput").ap()
    out_min = nc.dram_tensor("minaug", (128, PC_TILES), f32, kind="ExternalOutput").ap()

    with tile.TileContext(nc) as tc:
        with tc.tile_pool(name="const", bufs=1) as cpool, \
             tc.tile_pool(name="dram", bufs=1, space="DRAM") as dpool:

            # ---------------- load inputs ----------------
            g_sb = cpool.tile([H, C * W], f32)           # [32, 96] (y, (c,x))
            nc.sync.dma_start(
                out=g_sb[:].rearrange("y (c x) -> y c x", c=C),
                in_=grid.rearrange("c y x -> y c x"),
            )
            rm_sb = cpool.tile([H, OH], f32)             # [32, 94] = R^T
            nc.sync.dma_start(out=rm_sb[:], in_=rmat)
            aaug = cpool.tile([12, M_CORE], f16)         # host-built lhsT rows
            nc.sync.dma_start(out=aaug[:], in_=pcs)

            # ---------------- mesh refine on PE (fp32, exact) ----------------
            mstage = cpool.tile([OH, C * OH], f32)       # [94, 282] refined coords
            sqtmp = cpool.tile([OH, OH], f32)
            sq01 = cpool.tile([OH, OH], f32)
            sqsum = cpool.tile([OH, OH], f32)
            bhi = cpool.tile([OH, 4 * OH], f16)          # [94, 376]
            blo = cpool.tile([OH, 4 * OH], f16)
            a_sb = cpool.tile([H, C * OH], f32)          # [32x, (c,oh)]

            with tc.tile_pool(name="rpsum", bufs=2, space="PSUM") as rpool:
                for c in range(C):
                    pA = rpool.tile([H, OH], f32, name="pA")   # [x, oh]
                    nc.tensor.matmul(
                        out=pA[:],
                        lhsT=g_sb[:, c * W:(c + 1) * W],       # [y, x] ch c
                        rhs=rm_sb[:],
                        start=True, stop=True)
                    nc.vector.tensor_copy(a_sb[:, c * OH:(c + 1) * OH], pA[:])
                for c in range(C):
                    pB = rpool.tile([OH, OH], f32, name="pB")  # [oh, ow] ch c
                    nc.tensor.matmul(
                        out=pB[:],
                        lhsT=a_sb[:, c * OH:(c + 1) * OH],
                        rhs=rm_sb[:],
                        start=True, stop=True,
                    )
                    nc.vector.tensor_copy(mstage[:, c * OH:(c + 1) * OH], pB[:])

            # ---------------- ||m||^2 and fp16 hi/lo staging ----------------
            m0 = mstage[:, 0 * OH:1 * OH]
            m1 = mstage[:, 1 * OH:2 * OH]
            m2 = mstage[:, 2 * OH:3 * OH]
            sqtmp2 = cpool.tile([OH, OH], f32)
            nc.vector.tensor_tensor(out=sqtmp[:], in0=m0, in1=m0, op=MULT)
            nc.vector.tensor_tensor(out=sqtmp2[:], in0=m1, in1=m1, op=MULT)
            nc.vector.tensor_tensor(out=sq01[:], in0=sqtmp[:], in1=sqtmp2[:], op=ADD)
            nc.vector.tensor_tensor(out=sqtmp2[:], in0=m2, in1=m2, op=MULT)
            nc.vector.tensor_tensor(out=sqsum[:], in0=sq01[:], in1=sqtmp2[:], op=ADD)

            # coords: hi = f16(-2m), lo = f16(-2m - hi)
            for c in range(C):
                mc = mstage[:, c * OH:(c + 1) * OH]
                hc = bhi[:, c * OH:(c + 1) * OH]
                lc = blo[:, c * OH:(c + 1) * OH]
                nc.vector.tensor_scalar_mul(hc, mc, -2.0)
                nc.vector.scalar_tensor_tensor(
                    out=lc, in0=mc, scalar=-2.0, in1=hc, op0=MULT, op1=SUB)
            # ||m||^2: hi = f16(S), lo = f16(S - hi)
            hs = bhi[:, 3 * OH:4 * OH]
            ls = blo[:, 3 * OH:4 * OH]
            nc.vector.tensor_copy(hs, sqsum[:])
            nc.vector.scalar_tensor_tensor(
                out=ls, in0=sqsum[:], scalar=1.0, in1=hs, op0=MULT, op1=SUB)

            # ---------------- flatten via DRAM roundtrip ----------------
            dhi = dpool.tile([4, N_MESH], f16)
            dlo = dpool.tile([4, N_MESH], f16)
            nc.sync.dma_start(
                out=dhi[:].rearrange("c (h w) -> h c w", h=OH),
                in_=bhi[:].rearrange("h (c w) -> h c w", c=4),
            )
            nc.sync.dma_start(
                out=dlo[:].rearrange("c (h w) -> h c w", h=OH),
                in_=blo[:].rearrange("h (c w) -> h c w", c=4),
            )

            baug = cpool.tile([12, N_PAD], f16)
            nc.sync.dma_start(out=baug[0:4, 0:N_MESH], in_=dhi[:])
            nc.sync.dma_start(out=baug[4:8, 0:N_MESH], in_=dlo[:])
            nc.sync.dma_start(out=baug[8:12, 0:N_MESH], in_=dhi[:])
            nc.sync.dma_start(out=baug[:, N_MESH:N_PAD], in_=bpad)

            # ---------------- main loop ----------------
            results = cpool.tile([128, PC_TILES], f32)
            trash32 = cpool.tile([128, 2048], f32)
            trash16 = cpool.tile([128, 2048], f16)
            # 18 N-chunks per pc-tile = 4 quad-groups of 2048 + 1 pair of 1024.
            # Each group is min-reduced by ONE tensor_scalar(min, accum=min):
            # groups in DIRECT_QUADS reduce straight from PSUM (1 elem/cyc);
            # others are evacuated to SBUF fp16 by ScalarE, then reduced by
            # DVE in 4x packed mode.
            with tc.tile_pool(name="mpsum", bufs=2, space="PSUM") as mpool, \
                 tc.tile_pool(name="evac", bufs=3) as epool, \
                 tc.tile_pool(name="accp", bufs=2) as apool:
                for t in range(PC_TILES):
                    lh = aaug[:, t * 128:(t + 1) * 128]
                    accs = apool.tile([128, 5], f32, name="accs")
                    for q in range(5):
                        fd = 2048 if q < 4 else 1024
                        pd = mpool.tile([128, 2048], f32, name="pd")
                        for j in range(fd // 512):
                            ch = 4 * q + j
                            nc.tensor.matmul(
                                out=pd[:, j * 512:(j + 1) * 512], lhsT=lh,
                                rhs=baug[:, ch * 512:(ch + 1) * 512],
                                start=True, stop=True)
                        if q in DIRECT_QUADS:
                            nc.vector.tensor_scalar(
                                out=trash32[:, 0:fd], in0=pd[:, 0:fd],
                                scalar1=1e30, scalar2=None,
                                op0=MIN, op1=MIN,
                                accum_out=accs[:, q:q + 1])
                        else:
                            sb = epool.tile([128, 2048], f16, name="sb")
                            nc.scalar.copy(sb[:, 0:fd], pd[:, 0:fd])
                            nc.vector.tensor_scalar(
                                out=trash16[:, 0:fd], in0=sb[:, 0:fd],
                                scalar1=1e30, scalar2=None,
                                op0=MIN, op1=MIN,
                                accum_out=accs[:, q:q + 1])
                    nc.vector.tensor_reduce(
                        results[:, t:t + 1], accs[:],
                        axis=mybir.AxisListType.X, op=MIN)

            nc.sync.dma_start(out=out_min[:], in_=results[:])

    nc.compile()
    return nc


def _get_nc():
    if "nc" not in _BUILT:
        _BUILT["nc"] = _build_kernel()
    return _BUILT["nc"]


def _make_a_aug(pc_slice: np.ndarray) -> np.ndarray:
    """Host-side marshalling of pc slice [3, M] fp32 into the fp16 hi/lo
    augmented lhsT layout [12, M]: rows [p_hi, 1, p_hi, 1, p_lo, 0]."""
    m = pc_slice.shape[1]
    hi = pc_slice.astype(np.float16)
    lo = (pc_slice - hi.astype(np.float32)).astype(np.float16)
    a = np.zeros((12, m), dtype=np.float16)
    a[0:3] = hi
    a[3] = np.float16(1.0)
    a[4:7] = hi
    a[7] = np.float16(1.0)
    a[8:11] = lo
    a[11] = np.float16(0.0)
    return a


def _make_b_pad() -> np.ndarray:
    """Pad block for mesh columns N_MESH..N_PAD: zero coords, huge ||m||^2."""
    p = np.zeros((12, N_PAD - N_MESH), dtype=np.float16)
    p[3] = np.float16(PAD_BIG)   # b_hi ||m||^2 row
    p[11] = np.float16(PAD_BIG)  # b_hi dup ||m||^2 row
    return p


def kernel(network_mesh: np.ndarray, pc: np.ndarray) -> np.ndarray:
    global LAST_RESULTS
    from concourse.bass_utils import run_bass_kernel_spmd

    network_mesh = np.ascontiguousarray(network_mesh, dtype=np.float32)
    pc = np.ascontiguousarray(pc, dtype=np.float32)

    nc = _get_nc()
    rmat_t = np.ascontiguousarray(_interp_matrix().T)   # [32, 94]
    b_pad = _make_b_pad()

    in_maps = []
    for core in range(N_CORES):
        b, h = core // 2, core % 2
        in_maps.append({
            "mesh_grid": np.ascontiguousarray(network_mesh[b]),
            "a_aug": _make_a_aug(pc[b, :, h * M_CORE:(h + 1) * M_CORE]),
            "b_pad": b_pad,
            "rmat": rmat_t,
        })

    res = run_bass_kernel_spmd(nc, in_maps, core_ids=list(range(N_CORES)))
    LAST_RESULTS = res

    pnorm = np.sum(pc * pc, axis=1)                      # [B, 4096] fp32
    vals = []
    for core in range(N_CORES):
        b, h = core // 2, core % 2
        minaug = res.results[core]["minaug"]             # [128, 16]
        v = minaug.T.reshape(M_CORE)                     # point t*128+p order
        vals.append(v + pnorm[b, h * M_CORE:(h + 1) * M_CORE])
    dist2 = np.concatenate(vals)
    return np.float32(np.mean(dist2, dtype=np.float32))


# revision 27
# speedup vs baseline: 1.3232x; 1.3232x over previous
"""Trainium2 Bass kernel for nn_MeshLoss2D (chamfer min-distance mesh loss).

Computation: refine a (B,3,32,32) mesh grid by bilinear factor 3 to (B,3,94,94),
then for every point-cloud point (B,3,4096) find min squared distance to any
refined mesh point, and return the mean over all B*4096 points.

Sharding: 8 cores = (batch b, pc half h); each core handles 2048 pc points of
one batch and that batch's full mesh (8836 points, padded to 9216).

Device algorithm per core:
  - refine mesh via two fp32 matmuls with host-built interpolation matrix R^T
    (refined = Ry @ G_c @ Rx^T done as (G^T@Ry^T)^T stages on the PE).
  - build augmented fp16 hi/lo split operands:
      a = [p, 1]            (lhsT rows:  [a_hi, a_hi, a_lo]  -> 12 partitions)
      b = [-2m, ||m||^2]    (rhs  rows:  [b_hi, b_lo, b_hi])
    so   a.T b = -2 p.m + ||m||^2 = d(p,m) - ||p||^2   (exact to ~2^-22)
  - PE: for each 128-pc-point tile, stream 18 N-chunks (17x512 + 132 mesh
    points) into PSUM (fp32), grouped into 9 [128,1024] tiles.
  - min-reduce, split across two engines per group: groups in DIRECT_QUADS are
    tensor_reduce(min)'d straight from PSUM by the vector engine (1 elem/cyc);
    the rest are evacuated PSUM->SBUF fp16 by the scalar engine, then
    min-reduced by the vector engine in 4x packed mode via
    tensor_scalar(op0=min, accum_out, op1=min). Per-group partial mins land in
    accs[128,9]; one final tensor_reduce -> results[:, t].
Host: final = mean(minaug + ||p||^2)  (mean of ||p||^2 added back on host).
(Note: tensor_tensor_reduce and dual-PSUM-operand DVE ops crash/fail
 compilation on this stack - hence this ACT+DVE split scheme.)
"""

import os
import sys

for _p in ("/opt/trn_rl_repo", "/opt/trn_rl_repo/concourse"):
    if _p not in sys.path:
        sys.path.insert(0, _p)

import numpy as np

B, C, H, W = 4, 3, 32, 32
FACTOR = 3
OH = (H - 1) * FACTOR + 1        # 94
N_MESH = OH * OH                 # 8836
N_PAD = 9216                     # 18 chunks of 512
N_CHUNKS = N_PAD // 512          # 18
M_TOTAL = 4096
N_CORES = 8
M_CORE = M_TOTAL * B // N_CORES  # 2048 pc points per core
PC_TILES = M_CORE // 128         # 16
PAD_BIG = 60000.0                # sentinel ||m||^2 for padded mesh points
DIRECT_QUADS = {1, 4, 7}         # groups min-reduced directly from PSUM by DVE

_BUILT = {}
LAST_RESULTS = None


def _interp_matrix():
    """R [OH, H] fp32 with R[o, y0]=1-w, R[o, y0+1]=w replicating reference
    fp32 arithmetic (ys = arange(oh)/3 in fp32)."""
    ys = np.arange(OH, dtype=np.float32) / np.float32(FACTOR)
    y0 = np.clip(np.floor(ys).astype(np.int64), 0, H - 2)
    wy = ys - y0.astype(np.float32)
    R = np.zeros((OH, H), dtype=np.float32)
    R[np.arange(OH), y0] = np.float32(1.0) - wy
    R[np.arange(OH), y0 + 1] += wy
    return R


def _build_kernel(direct_quads=None, psum_bufs=4, evac_bufs=8, quad_width=1024,
                  group_order=None, alt_direct=None, acc_bufs=2):
    from concourse import bacc, mybir
    import concourse.tile as tile

    if direct_quads is None:
        direct_quads = DIRECT_QUADS

    f32 = mybir.dt.float32
    f16 = mybir.dt.float16
    MIN = mybir.AluOpType.min
    MULT = mybir.AluOpType.mult
    SUB = mybir.AluOpType.subtract
    ADD = mybir.AluOpType.add

    nc = bacc.Bacc(
        "TRN2",
        target_bir_lowering=False,
        debug=False,
        enable_asserts=False,
        num_devices=N_CORES,
    )

    grid = nc.dram_tensor("mesh_grid", (C, H, W), f32, kind="ExternalInput").ap()
    pcs = nc.dram_tensor("a_aug", (12, M_CORE), f16, kind="ExternalInput").ap()
    rmat = nc.dram_tensor("rmat", (H, OH), f32, kind="ExternalInput").ap()
    out_min = nc.dram_tensor("minaug", (128, PC_TILES), f32, kind="ExternalOutput").ap()

    with tile.TileContext(nc) as tc:
        with tc.tile_pool(name="const", bufs=1) as cpool, \
             tc.tile_pool(name="dram", bufs=1, space="DRAM") as dpool:

            # ---------------- load inputs ----------------
            g_sb = cpool.tile([H, C * W], f32)           # [32, 96] (y, (c,x))
            nc.sync.dma_start(
                out=g_sb[:].rearrange("y (c x) -> y c x", c=C),
                in_=grid.rearrange("c y x -> y c x"),
            )
            rm_sb = cpool.tile([H, OH], f32)             # [32, 94] = R^T
            nc.sync.dma_start(out=rm_sb[:], in_=rmat)
            aaug = cpool.tile([12, M_CORE], f16)         # host-built lhsT rows
            nc.sync.dma_start(out=aaug[:], in_=pcs)

            # ---------------- mesh refine on PE (fp32, exact) ----------------
            mstage = cpool.tile([OH, C * OH], f32)       # [94, 282] refined coords
            sqtmp = cpool.tile([OH, OH], f32)
            sq01 = cpool.tile([OH, OH], f32)
            sqsum = cpool.tile([OH, OH], f32)
            bhi = cpool.tile([OH, 4 * OH], f16)          # [94, 376]
            blo = cpool.tile([OH, 4 * OH], f16)
            a_sb = cpool.tile([H, C * OH], f32)          # [32x, (c,oh)]

            with tc.tile_pool(name="rpsum", bufs=2, space="PSUM") as rpool:
                for c in range(C):
                    pA = rpool.tile([H, OH], f32, name="pA")   # [x, oh]
                    nc.tensor.matmul(
                        out=pA[:],
                        lhsT=g_sb[:, c * W:(c + 1) * W],       # [y, x] ch c
                        rhs=rm_sb[:],
                        start=True, stop=True)
                    nc.vector.tensor_copy(a_sb[:, c * OH:(c + 1) * OH], pA[:])
                for c in range(C):
                    pB = rpool.tile([OH, OH], f32, name="pB")  # [oh, ow] ch c
                    nc.tensor.matmul(
                        out=pB[:],
                        lhsT=a_sb[:, c * OH:(c + 1) * OH],
                        rhs=rm_sb[:],
                        start=True, stop=True,
                    )
                    nc.vector.tensor_copy(mstage[:, c * OH:(c + 1) * OH], pB[:])

            # ---------------- ||m||^2 and fp16 hi/lo staging ----------------
            m0 = mstage[:, 0 * OH:1 * OH]
            m1 = mstage[:, 1 * OH:2 * OH]
            m2 = mstage[:, 2 * OH:3 * OH]
            sqtmp2 = cpool.tile([OH, OH], f32)
            nc.vector.tensor_tensor(out=sqtmp[:], in0=m0, in1=m0, op=MULT)
            nc.vector.tensor_tensor(out=sqtmp2[:], in0=m1, in1=m1, op=MULT)
            nc.vector.tensor_tensor(out=sq01[:], in0=sqtmp[:], in1=sqtmp2[:], op=ADD)
            nc.vector.tensor_tensor(out=sqtmp2[:], in0=m2, in1=m2, op=MULT)
            nc.vector.tensor_tensor(out=sqsum[:], in0=sq01[:], in1=sqtmp2[:], op=ADD)

            # coords: hi = f16(-2m), lo = f16(-2m - hi)
            for c in range(C):
                mc = mstage[:, c * OH:(c + 1) * OH]
                hc = bhi[:, c * OH:(c + 1) * OH]
                lc = blo[:, c * OH:(c + 1) * OH]
                nc.vector.tensor_scalar_mul(hc, mc, -2.0)
                nc.vector.scalar_tensor_tensor(
                    out=lc, in0=mc, scalar=-2.0, in1=hc, op0=MULT, op1=SUB)
            # ||m||^2: hi = f16(S), lo = f16(S - hi)
            hs = bhi[:, 3 * OH:4 * OH]
            ls = blo[:, 3 * OH:4 * OH]
            nc.vector.tensor_copy(hs, sqsum[:])
            nc.vector.scalar_tensor_tensor(
                out=ls, in0=sqsum[:], scalar=1.0, in1=hs, op0=MULT, op1=SUB)

            # ---------------- flatten via DRAM roundtrip ----------------
            dhi = dpool.tile([4, N_MESH], f16)
            dlo = dpool.tile([4, N_MESH], f16)
            nc.sync.dma_start(
                out=dhi[:].rearrange("c (h w) -> h c w", h=OH),
                in_=bhi[:].rearrange("h (c w) -> h c w", c=4),
            )
            nc.sync.dma_start(
                out=dlo[:].rearrange("c (h w) -> h c w", h=OH),
                in_=blo[:].rearrange("h (c w) -> h c w", c=4),
            )

            baug = cpool.tile([12, N_PAD], f16)
            nc.sync.dma_start(out=baug[0:4, 0:N_MESH], in_=dhi[:])
            nc.sync.dma_start(out=baug[4:8, 0:N_MESH], in_=dlo[:])
            nc.sync.dma_start(out=baug[8:12, 0:N_MESH], in_=dhi[:])

            # ---------------- main loop ----------------
            results = cpool.tile([128, PC_TILES], f32)
            trash16 = cpool.tile([128, 2048], f16)
            # 18 N-chunks per pc-tile = 4 quad-groups of 2048 + 1 pair of 1024.
            # Each group is min-reduced by ONE tensor_scalar(min, accum=min):
            # groups in DIRECT_QUADS reduce straight from PSUM (1 elem/cyc);
            # others are evacuated to SBUF fp16 by ScalarE, then reduced by
            # DVE in 4x packed mode.
            # Partition the 18 N-chunks of each pc-tile into groups; each
            # group is one PSUM tile and one reduce op (direct from PSUM on
            # DVE at 1x, or ScalarE-evacuated to SBUF fp16 then DVE at 4x).
            QW = quad_width // 512                  # chunks per full group
            n_groups = (N_CHUNKS + QW - 1) // QW
            LAST_W = N_MESH - 512 * (N_CHUNKS - 1)  # 132: final partial chunk
            with tc.tile_pool(name="mpsum", bufs=psum_bufs, space="PSUM") as mpool, \
                 tc.tile_pool(name="evac", bufs=evac_bufs) as epool, \
                 tc.tile_pool(name="accp", bufs=acc_bufs) as apool:
                for t in range(PC_TILES):
                    lh = aaug[:, t * 128:(t + 1) * 128]
                    accs = apool.tile([128, n_groups], f32, name="accs")
                    order = group_order if group_order else range(n_groups)
                    for q in order:
                        ch0 = q * QW
                        nch = min(QW, N_CHUNKS - ch0)
                        fd = 0
                        pd = mpool.tile([128, quad_width], f32, name="pd")
                        for j in range(nch):
                            ch = ch0 + j
                            w = LAST_W if ch == N_CHUNKS - 1 else 512
                            nc.tensor.matmul(
                                out=pd[:, fd:fd + w], lhsT=lh,
                                rhs=baug[:, ch * 512:ch * 512 + w],
                                start=True, stop=True)
                            fd += w
                        if q in direct_quads or (alt_direct and t % 2 == 1 and q in alt_direct):
                            nc.vector.tensor_reduce(
                                accs[:, q:q + 1], pd[:, 0:fd],
                                axis=mybir.AxisListType.X, op=MIN)
                        else:
                            sb = epool.tile([128, quad_width], f16, name="sb")
                            nc.scalar.copy(sb[:, 0:fd], pd[:, 0:fd])
                            nc.vector.tensor_scalar(
                                out=trash16[:, 0:fd], in0=sb[:, 0:fd],
                                scalar1=1e30, scalar2=None,
                                op0=MIN, op1=MIN,
                                accum_out=accs[:, q:q + 1])
                    nc.vector.tensor_reduce(
                        results[:, t:t + 1], accs[:],
                        axis=mybir.AxisListType.X, op=MIN)

            nc.sync.dma_start(out=out_min[:], in_=results[:])

    nc.compile()
    return nc


def _get_nc():
    if "nc" not in _BUILT:
        _BUILT["nc"] = _build_kernel()
    return _BUILT["nc"]


def _make_a_aug(pc_slice: np.ndarray) -> np.ndarray:
    """Host-side marshalling of pc slice [3, M] fp32 into the fp16 hi/lo
    augmented lhsT layout [12, M]: rows [p_hi, 1, p_hi, 1, p_lo, 0]."""
    m = pc_slice.shape[1]
    hi = pc_slice.astype(np.float16)
    lo = (pc_slice - hi.astype(np.float32)).astype(np.float16)
    a = np.zeros((12, m), dtype=np.float16)
    a[0:3] = hi
    a[3] = np.float16(1.0)
    a[4:7] = hi
    a[7] = np.float16(1.0)
    a[8:11] = lo
    a[11] = np.float16(0.0)
    return a


def kernel(network_mesh: np.ndarray, pc: np.ndarray) -> np.ndarray:
    global LAST_RESULTS
    from concourse.bass_utils import run_bass_kernel_spmd

    network_mesh = np.ascontiguousarray(network_mesh, dtype=np.float32)
    pc = np.ascontiguousarray(pc, dtype=np.float32)

    nc = _get_nc()
    rmat_t = np.ascontiguousarray(_interp_matrix().T)   # [32, 94]

    in_maps = []
    for core in range(N_CORES):
        b, h = core // 2, core % 2
        in_maps.append({
            "mesh_grid": np.ascontiguousarray(network_mesh[b]),
            "a_aug": _make_a_aug(pc[b, :, h * M_CORE:(h + 1) * M_CORE]),
            "rmat": rmat_t,
        })

    res = run_bass_kernel_spmd(nc, in_maps, core_ids=list(range(N_CORES)))
    LAST_RESULTS = res

    pnorm = np.sum(pc * pc, axis=1)                      # [B, 4096] fp32
    vals = []
    for core in range(N_CORES):
        b, h = core // 2, core % 2
        minaug = res.results[core]["minaug"]             # [128, 16]
        v = minaug.T.reshape(M_CORE)                     # point t*128+p order
        vals.append(v + pnorm[b, h * M_CORE:(h + 1) * M_CORE])
    dist2 = np.concatenate(vals)
    return np.array(np.mean(dist2, dtype=np.float32), dtype=np.float32)
